# revision 6
# baseline (speedup 1.0000x reference)
"""ConvSTFT on Trainium2: strided conv of x[32, 480000] against a fixed
[514, 1, 400] Fourier basis, hop 100 -> out [32, 514, 4803] f32.

Sharding: pure data parallel. Batch dim (32) split 4-per-core across 8
NeuronCores; the small weight is replicated.

Split of work: the PE matmul cost is (#streams) x N cycles, with
#streams = ceil(C/128) * ceil(WIN/HOP) per frame-column. C=514 needs 5
channel tiles, the 5th holding only 2 channels -- 25% wasted PE time.
So the device computes only channels 0..511 (4 full tiles, the bf16 PE
floor of 16 streams/frame-column ~= 128us/core), while the host computes
the last 2 channels with one small BLAS GEMM over the strided frame view
(<1% of the FLOPs). Device output is stored bf16 (halves the dominant
output DMA: 39.3 -> 19.7 MB/core) and upcast on the host.

Host prep (sharding layer): pad x by 300 on both sides, then transpose
per batch to XT[b, r, f'] = x_padded[b, 100 f' + r] so device input DMAs
move whole [100, cols] panels with multi-KB contiguous lines. The weight
is passed as wt[j, r, c] = weight[c, 0, 100 j + r] so each j-plane is an
independent [100, 512] panel. Both are cast to bf16.

Per-core device kernel (Bass/Tile):
  t = 100j + r decomposition (j in 0..3, r in 0..99) turns the overlapped
  conv into 4 PSUM-accumulated matmuls:
      out[c, f] = sum_j sum_r wt[j, r, c] * XT[r, f + j]
  - lhsT = wt[j, r, c-tile] (K=100, M=128), rhs = XT[r, f-tile] (N<=512),
    fp32 PSUM accumulation over j, all 8 PSUM banks in flight.
  - Frame tiles are the OUTER loop (channel tiles inner), so early compute
    only needs the first 516 input columns and input-panel deadlines are
    4x looser than channel-outer order.
  - PSUM evacuated alternately by DVE/ACT into SBUF rows [128, 4803]
    bf16 (cast on copy), stored in four column pieces per (batch,
    channel-tile) so the post-compute drain is four tiny ~50 KB DMAs.
  - Startup: engines other than PE and SP(sync) take 5-8 us to boot, so
    every critical first load (4 weight j-planes + first XT panel) is
    issued on the sync ring, which boots immediately. The PE clock-gate
    warmup matmuls read the first weight plane (no memset needed), so PE
    ramps as soon as that ~100 KB DMA lands. Later XT panels go to the
    scalar ring, whose ~7 us boot is hidden by the relaxed deadlines.
"""

import numpy as np
import ml_dtypes

WIN, HOP, C = 400, 100, 514
C_DEV = 512                           # channels computed on device
B, T = 32, 480000
PAD = WIN - HOP                       # 300
N_CORES = 8
B_LOC = B // N_CORES                  # 4
T_PAD = T + 2 * PAD                   # 480600
N_FRAMES = (T_PAD - WIN) // HOP + 1   # 4803
N_CHUNKS = 4864                       # padded frame columns (128-aligned)
NJ = WIN // HOP                       # 4

F_TILE = 512
C_TILE = 128
FIRST_COLS = 516                      # first XT panel (critical load)
B0_PANEL = 1088                       # later b0 panels (sync ring)
BN_PANEL = 1216                       # panels for batches 1.. (scalar ring)
N_WARMUP = 8
STORE_EVERY = 3                       # ftile groups per output store piece


def build_program(b_loc=B_LOC, n_chunks=N_CHUNKS, n_frames=N_FRAMES):
    import concourse.bacc as bacc
    import concourse.mybir as mybir
    import concourse.tile as tile

    dt = mybir.dt
    assert n_frames + NJ - 1 <= n_chunks

    nc = bacc.Bacc("TRN2", target_bir_lowering=False, debug=False)
    x_d = nc.dram_tensor(
        "x", [b_loc, HOP, n_chunks], dt.bfloat16, kind="ExternalInput"
    ).ap()
    w_d = nc.dram_tensor(
        "wt", [NJ, HOP, C_DEV], dt.bfloat16, kind="ExternalInput"
    ).ap()
    o_d = nc.dram_tensor(
        "out", [b_loc, C_DEV, n_frames], dt.bfloat16, kind="ExternalOutput"
    ).ap()

    ctiles = [(c0, min(C_TILE, C_DEV - c0)) for c0 in range(0, C_DEV, C_TILE)]
    ftiles = [(f0, min(F_TILE, n_frames - f0)) for f0 in range(0, n_frames, F_TILE)]
    n_ft = len(ftiles)
    # store piece boundaries: after ftile group fi, store cols [lo, hi)
    store_at = {}
    lo = 0
    for fi in range(STORE_EVERY - 1, n_ft - 1, STORE_EVERY):
        hi = ftiles[fi][0] + ftiles[fi][1]
        store_at[fi] = (lo, hi)
        lo = hi
    store_at[n_ft - 1] = (lo, n_frames)

    with tile.TileContext(nc) as tc:
        with (
            tc.tile_pool(name="const", bufs=1) as constp,
            tc.tile_pool(name="xt", bufs=2) as xtp,
            tc.tile_pool(name="orow", bufs=8) as orowp,
            tc.tile_pool(name="mmps", bufs=8, space="PSUM") as mmps,
        ):
            # Critical first loads, all on the fast-booting sync ring.
            wsb = constp.tile([HOP, NJ, C_DEV], dt.bfloat16)
            for j in range(NJ):
                nc.sync.dma_start(wsb[:, j, :], w_d[j])
            xt0 = xtp.tile([HOP, n_chunks], dt.bfloat16, tag="xt")
            nc.sync.dma_start(xt0[:, 0:FIRST_COLS], x_d[0, :, 0:FIRST_COLS])
            for g0 in range(FIRST_COLS, n_chunks, B0_PANEL):
                gs = min(B0_PANEL, n_chunks - g0)
                nc.sync.dma_start(xt0[:, g0 : g0 + gs], x_d[0, :, g0 : g0 + gs])

            # Warm the PE clock gate (HAM) with throwaway matmuls reading the
            # first weight plane (needs ~3.4us of sustained PE activity to
            # lift the clock from 1.2 to 2.4 GHz).
            wps = mmps.tile([128, F_TILE], dt.float32, tag="ps")
            for _ in range(N_WARMUP):
                nc.tensor.matmul(wps[0:16, :], wsb[0:HOP, 0, 0:16], wsb[0:HOP, 0, :])

            ncopy = 0

            def mm_group(xt, orow, c0, cm, f0, fn):
                nonlocal ncopy
                ps = mmps.tile([128, F_TILE], dt.float32, tag="ps")
                for j in range(NJ):
                    nc.tensor.matmul(
                        ps[0:cm, 0:fn],
                        wsb[0:HOP, j, c0 : c0 + cm],
                        xt[0:HOP, f0 + j : f0 + j + fn],
                        start=(j == 0),
                        stop=(j == NJ - 1),
                    )
                # alternate evacuation between DVE and ACT (casts f32->bf16)
                if ncopy % 2 == 1:
                    nc.scalar.copy(orow[0:cm, f0 : f0 + fn], ps[0:cm, 0:fn])
                else:
                    nc.vector.tensor_copy(orow[0:cm, f0 : f0 + fn], ps[0:cm, 0:fn])
                ncopy += 1

            for b in range(b_loc):
                if b == 0:
                    xt = xt0
                else:
                    # later batches on the scalar ring: they queue behind each
                    # other and cannot starve the critical sync-ring loads
                    xt = xtp.tile([HOP, n_chunks], dt.bfloat16, tag="xt")
                    for g0 in range(0, n_chunks, BN_PANEL):
                        gs = min(BN_PANEL, n_chunks - g0)
                        nc.scalar.dma_start(
                            xt[:, g0 : g0 + gs], x_d[b, :, g0 : g0 + gs]
                        )

                orows = [
                    orowp.tile(
                        [128, n_frames], dt.bfloat16, tag="orow", name=f"orow_{b}_{ci}"
                    )
                    for ci in range(len(ctiles))
                ]
                for fi, (f0, fn) in enumerate(ftiles):
                    for ci, (c0, cm) in enumerate(ctiles):
                        mm_group(xt, orows[ci], c0, cm, f0, fn)
                        if fi in store_at:
                            slo, shi = store_at[fi]
                            nc.sync.dma_start(
                                o_d[b, c0 : c0 + cm, slo:shi],
                                orows[ci][0:cm, slo:shi],
                            )

    nc.compile()
    return nc


_NC = None
LAST_RESULTS = None


def _ensure_axon_hooks_stub():
    """If BASS_TRACE is set but the container's antenv lacks axon_hooks,
    run_bass_kernel_spmd would crash on import; degrade to no-trace."""
    import sys

    try:
        import antenv.axon_hooks  # noqa: F401
    except ImportError:
        import types

        import antenv

        m = types.ModuleType("antenv.axon_hooks")
        m.get_axon_ntff_profile_hook = lambda: None
        m.set_axon_ntff_profile_hook = lambda h: None
        sys.modules["antenv.axon_hooks"] = m
        antenv.axon_hooks = m


def _prep_inputs(x, weight):
    x = np.asarray(x, dtype=np.float32)
    w = np.asarray(weight, dtype=np.float32)
    nb = x.shape[0]
    xp = np.zeros((nb, N_CHUNKS * HOP), dtype=np.float32)
    xp[:, PAD : PAD + x.shape[1]] = x
    # full per-batch transpose: xdev[b, r, f'] = xp[b, 100 f' + r]
    xdev = np.ascontiguousarray(
        xp.reshape(nb, N_CHUNKS, HOP).transpose(0, 2, 1)
    ).astype(ml_dtypes.bfloat16)
    # wt[j, r, c] = weight[c, 0, 100 j + r]
    wt = np.ascontiguousarray(
        w.reshape(C, WIN)[:C_DEV].T.reshape(NJ, HOP, C_DEV)
    ).astype(ml_dtypes.bfloat16)
    return xp, xdev, wt


def _host_tail_channels(xp, w):
    """Channels C_DEV..C-1 via one BLAS GEMM over the strided frame view."""
    w2 = np.ascontiguousarray(
        np.asarray(w, dtype=np.float32).reshape(C, WIN)[C_DEV:].T
    )  # [WIN, C - C_DEV]
    v = np.lib.stride_tricks.sliding_window_view(xp, WIN, axis=1)[:, ::HOP, :]
    v = v[:, :N_FRAMES]  # [B, N_FRAMES, WIN]
    out2 = np.tensordot(v, w2, axes=([2], [0]))  # [B, N_FRAMES, C-C_DEV]
    return np.ascontiguousarray(out2.transpose(0, 2, 1))


def kernel(x, weight):
    global _NC, LAST_RESULTS
    from concourse.bass_utils import run_bass_kernel_spmd

    _ensure_axon_hooks_stub()
    xp, xdev, wt = _prep_inputs(x, weight)
    tail = _host_tail_channels(xp, weight)
    if _NC is None:
        _NC = build_program()
    in_maps = [
        {"x": np.ascontiguousarray(xdev[c * B_LOC : (c + 1) * B_LOC]), "wt": wt}
        for c in range(N_CORES)
    ]
    res = run_bass_kernel_spmd(_NC, in_maps, core_ids=list(range(N_CORES)))
    LAST_RESULTS = res
    out = np.empty((B, C, N_FRAMES), dtype=np.float32)
    for c in range(N_CORES):
        out[c * B_LOC : (c + 1) * B_LOC, :C_DEV] = res.results[c]["out"]
    out[:, C_DEV:] = tail
    return out


# revision 7
# speedup vs baseline: 1.2065x; 1.2065x over previous
"""ConvSTFT on Trainium2: strided conv of x[32, 480000] against a fixed
[514, 1, 400] Fourier basis, hop 100 -> out [32, 514, 4803] f32.

Sharding: pure data parallel. Batch dim (32) split 4-per-core across 8
NeuronCores; the small weight is replicated.

Split of work: the PE matmul cost is (#streams) x N cycles, with
#streams = ceil(C/128) * ceil(WIN/HOP) per frame-column. C=514 needs 5
channel tiles, the 5th holding only 2 channels -- 25% wasted PE time.
So the device computes only channels 0..511 (4 full tiles, the bf16 PE
floor of 16 streams/frame-column ~= 128us/core), while the host computes
the last 2 channels with one small BLAS GEMM over the strided frame view
(<1% of the FLOPs). Device output is stored bf16 (halves the dominant
output DMA: 39.3 -> 19.7 MB/core) and upcast on the host.

Host prep (sharding layer): pad x by 300 on both sides, then transpose
per batch to XT[b, r, f'] = x_padded[b, 100 f' + r] so device input DMAs
move whole [100, cols] panels with multi-KB contiguous lines. The weight
is passed as wt[j, r, c] = weight[c, 0, 100 j + r] so each j-plane is an
independent [100, 512] panel. Both are cast to bf16.

Per-core device kernel (Bass/Tile):
  t = 100j + r decomposition (j in 0..3, r in 0..99) turns the overlapped
  conv into 4 PSUM-accumulated matmuls:
      out[c, f] = sum_j sum_r wt[j, r, c] * XT[r, f + j]
  - lhsT = wt[j, r, c-tile] (K=100, M=128), rhs = XT[r, f-tile] (N<=512),
    fp32 PSUM accumulation over j, all 8 PSUM banks in flight.
  - Frame tiles are the OUTER loop (channel tiles inner), so early compute
    only needs the first 516 input columns and input-panel deadlines are
    4x looser than channel-outer order.
  - PSUM evacuated alternately by DVE/ACT into SBUF rows [128, 4803]
    bf16 (cast on copy), stored in four column pieces per (batch,
    channel-tile) so the post-compute drain is four tiny ~50 KB DMAs.
  - Startup: engines other than PE and SP(sync) take 5-8 us to boot, so
    every critical first load (4 weight j-planes + first XT panel) is
    issued on the sync ring, which boots immediately. The PE clock-gate
    warmup matmuls read the first weight plane (no memset needed), so PE
    ramps as soon as that ~100 KB DMA lands. Later XT panels go to the
    scalar ring, whose ~7 us boot is hidden by the relaxed deadlines.
"""

import numpy as np
import ml_dtypes

WIN, HOP, C = 400, 100, 514
C_DEV = 512                           # channels computed on device
B, T = 32, 480000
PAD = WIN - HOP                       # 300
N_CORES = 8
B_LOC = B // N_CORES                  # 4
T_PAD = T + 2 * PAD                   # 480600
N_FRAMES = (T_PAD - WIN) // HOP + 1   # 4803
N_CHUNKS = 4864                       # padded frame columns (128-aligned)
NJ = WIN // HOP                       # 4

F_TILE = 512
C_TILE = 128
FIRST_COLS = 516                      # first XT panel (critical load)
B0_PANEL = 1088                       # later b0 panels (sync ring)
BN_PANEL = 1216                       # panels for batches 1.. (scalar ring)
N_WARMUP = 8
STORE_EVERY = 3                       # ftile groups per output store piece


def build_program(b_loc=B_LOC, n_chunks=N_CHUNKS, n_frames=N_FRAMES):
    import concourse.bacc as bacc
    import concourse.mybir as mybir
    import concourse.tile as tile

    dt = mybir.dt
    assert n_frames + NJ - 1 <= n_chunks

    nc = bacc.Bacc("TRN2", target_bir_lowering=False, debug=False)
    x_d = nc.dram_tensor(
        "x", [b_loc, 128, n_chunks], dt.bfloat16, kind="ExternalInput"
    ).ap()
    w_d = nc.dram_tensor(
        "wt", [NJ, HOP, C_DEV], dt.bfloat16, kind="ExternalInput"
    ).ap()
    o_d = nc.dram_tensor(
        "out", [b_loc, C_DEV, n_frames], dt.bfloat16, kind="ExternalOutput"
    ).ap()

    ctiles = [(c0, min(C_TILE, C_DEV - c0)) for c0 in range(0, C_DEV, C_TILE)]
    ftiles = [(f0, min(F_TILE, n_frames - f0)) for f0 in range(0, n_frames, F_TILE)]
    n_ft = len(ftiles)
    # store piece boundaries: after ftile group fi, store cols [lo, hi)
    store_at = {}
    lo = 0
    for fi in range(STORE_EVERY - 1, n_ft - 1, STORE_EVERY):
        hi = ftiles[fi][0] + ftiles[fi][1]
        store_at[fi] = (lo, hi)
        lo = hi
    store_at[n_ft - 1] = (lo, n_frames)

    with tile.TileContext(nc) as tc:
        with (
            tc.tile_pool(name="const", bufs=1) as constp,
            tc.tile_pool(name="xt", bufs=2) as xtp,
            tc.tile_pool(name="orow", bufs=8) as orowp,
            tc.tile_pool(name="mmps", bufs=8, space="PSUM") as mmps,
        ):
            # Critical first loads, all on the fast-booting sync ring.
            xt0 = xtp.tile([128, n_chunks], dt.bfloat16, tag="xt")
            nc.sync.dma_start(xt0[:, 0:FIRST_COLS], x_d[0, :, 0:FIRST_COLS])
            wsb = constp.tile([HOP, NJ, C_DEV], dt.bfloat16)
            for j in range(NJ):
                nc.sync.dma_start(wsb[:, j, :], w_d[j])
            for g0 in range(FIRST_COLS, n_chunks, B0_PANEL):
                gs = min(B0_PANEL, n_chunks - g0)
                nc.sync.dma_start(xt0[:, g0 : g0 + gs], x_d[0, :, g0 : g0 + gs])

            # Warm the PE clock gate (HAM) with throwaway matmuls reading the
            # first weight plane (needs ~3.4us of sustained PE activity to
            # lift the clock from 1.2 to 2.4 GHz).
            wps = mmps.tile([128, F_TILE], dt.float32, tag="ps")
            for _ in range(N_WARMUP):
                nc.tensor.matmul(wps[0:16, :], xt0[:, 0:16], xt0[:, 0:F_TILE])

            ncopy = 0

            def mm_group(xt, orow, c0, cm, f0, fn):
                nonlocal ncopy
                ps = mmps.tile([128, F_TILE], dt.float32, tag="ps")
                for j in range(NJ):
                    nc.tensor.matmul(
                        ps[0:cm, 0:fn],
                        wsb[0:HOP, j, c0 : c0 + cm],
                        xt[0:HOP, f0 + j : f0 + j + fn],
                        start=(j == 0),
                        stop=(j == NJ - 1),
                    )
                # alternate evacuation between DVE and ACT (casts f32->bf16)
                if ncopy % 2 == 1:
                    nc.scalar.copy(orow[0:cm, f0 : f0 + fn], ps[0:cm, 0:fn])
                else:
                    nc.vector.tensor_copy(orow[0:cm, f0 : f0 + fn], ps[0:cm, 0:fn])
                ncopy += 1

            for b in range(b_loc):
                if b == 0:
                    xt = xt0
                else:
                    # later batches on the scalar ring: they queue behind each
                    # other and cannot starve the critical sync-ring loads
                    xt = xtp.tile([128, n_chunks], dt.bfloat16, tag="xt")
                    for g0 in range(0, n_chunks, BN_PANEL):
                        gs = min(BN_PANEL, n_chunks - g0)
                        nc.scalar.dma_start(
                            xt[:, g0 : g0 + gs], x_d[b, :, g0 : g0 + gs]
                        )

                orows = [
                    orowp.tile(
                        [128, n_frames], dt.bfloat16, tag="orow", name=f"orow_{b}_{ci}"
                    )
                    for ci in range(len(ctiles))
                ]
                for fi, (f0, fn) in enumerate(ftiles):
                    for ci, (c0, cm) in enumerate(ctiles):
                        mm_group(xt, orows[ci], c0, cm, f0, fn)
                        if fi in store_at:
                            slo, shi = store_at[fi]
                            nc.sync.dma_start(
                                o_d[b, c0 : c0 + cm, slo:shi],
                                orows[ci][0:cm, slo:shi],
                            )

    nc.compile()
    return nc


_NC = None
LAST_RESULTS = None


def _ensure_axon_hooks_stub():
    """If BASS_TRACE is set but the container's antenv lacks axon_hooks,
    run_bass_kernel_spmd would crash on import; degrade to no-trace."""
    import sys

    try:
        import antenv.axon_hooks  # noqa: F401
    except ImportError:
        import types

        import antenv

        m = types.ModuleType("antenv.axon_hooks")
        m.get_axon_ntff_profile_hook = lambda: None
        m.set_axon_ntff_profile_hook = lambda h: None
        sys.modules["antenv.axon_hooks"] = m
        antenv.axon_hooks = m


def _prep_inputs(x, weight):
    x = np.asarray(x, dtype=np.float32)
    w = np.asarray(weight, dtype=np.float32)
    nb = x.shape[0]
    xp = np.zeros((nb, N_CHUNKS * HOP), dtype=np.float32)
    xp[:, PAD : PAD + x.shape[1]] = x
    # full per-batch transpose: xdev[b, r, f'] = xp[b, 100 f' + r], padded
    # to 128 rows of zeros (rows 100..127) so the PE warmup has K=128 data
    xdev = np.zeros((nb, 128, N_CHUNKS), dtype=ml_dtypes.bfloat16)
    xdev[:, :HOP] = xp.reshape(nb, N_CHUNKS, HOP).transpose(0, 2, 1).astype(
        ml_dtypes.bfloat16
    )
    # wt[j, r, c] = weight[c, 0, 100 j + r]
    wt = np.ascontiguousarray(
        w.reshape(C, WIN)[:C_DEV].T.reshape(NJ, HOP, C_DEV)
    ).astype(ml_dtypes.bfloat16)
    return xp, xdev, wt


def _host_tail_channels(xp, w):
    """Channels C_DEV..C-1 via one BLAS GEMM over the strided frame view."""
    w2 = np.ascontiguousarray(
        np.asarray(w, dtype=np.float32).reshape(C, WIN)[C_DEV:].T
    )  # [WIN, C - C_DEV]
    v = np.lib.stride_tricks.sliding_window_view(xp, WIN, axis=1)[:, ::HOP, :]
    v = v[:, :N_FRAMES]  # [B, N_FRAMES, WIN]
    out2 = np.tensordot(v, w2, axes=([2], [0]))  # [B, N_FRAMES, C-C_DEV]
    return np.ascontiguousarray(out2.transpose(0, 2, 1))


def kernel(x, weight):
    global _NC, LAST_RESULTS
    from concourse.bass_utils import run_bass_kernel_spmd

    _ensure_axon_hooks_stub()
    xp, xdev, wt = _prep_inputs(x, weight)
    tail = _host_tail_channels(xp, weight)
    if _NC is None:
        _NC = build_program()
    in_maps = [
        {"x": np.ascontiguousarray(xdev[c * B_LOC : (c + 1) * B_LOC]), "wt": wt}
        for c in range(N_CORES)
    ]
    res = run_bass_kernel_spmd(_NC, in_maps, core_ids=list(range(N_CORES)))
    LAST_RESULTS = res
    out = np.empty((B, C, N_FRAMES), dtype=np.float32)
    for c in range(N_CORES):
        out[c * B_LOC : (c + 1) * B_LOC, :C_DEV] = res.results[c]["out"]
    out[:, C_DEV:] = tail
    return out
